# revision 21
# baseline (speedup 1.0000x reference)
"""DeformConvTranspose2d Bass kernel for 8 Trainium2 NeuronCores — v3.

Data-parallel over batch (one batch element per core).

Per core:
  Pass A (dense GEMM): V[pixel, tap, cout] = x_col @ W, computed on 32
  pixel-tiles of 128 (no bucket padding), written to DRAM VD [36864, 256].
  Pass B (scatter): entries (pixel, tap) bucketed by output row index
  bi = floor(y)+1 (mixed taps, capacity 384 = 3 K-tiles).  dma_gather
  (transpose=False, 4 buckets per call, per-group window base keeps the
  int16 indices small) pulls V rows slot-on-partition.  A custom fused
  DVE op builds lhsT = relu(1 - |iota - xpa|) * wy per (tile, corner);
  out_psum[ox 128, cout 256] accumulates 3 matmuls from bucket r
  (corner dy=1) + 3 from bucket r+1 (corner dy=0) + rank-1 bias matmul,
  then ACT copies to bf16 and DMA writes out[r] of [oy, ox, cout].

Host: prep bucketization (vectorized numpy), final transpose to
[cout, oy, ox] f32.
"""

import os
import re
import sys

sys.path.insert(0, "/opt/trn_rl_repo")

import numpy as np

from concourse import bass, mybir, library_config
from concourse.library_overlay import lower_extended_insts
import concourse.tile as tile

BF16 = mybir.dt.np(mybir.dt.bfloat16)

# problem constants (hardcoded per contract)
B = 8
CIN = 256
COUT = 256
H = W = 64
HW = H * W
NK = 9
KH = KW = 3
STRIDE, PAD, OUT_PAD, DIL = 2, 1, 1, 1
OHH = (H - 1) * STRIDE - 2 * PAD + DIL * (KH - 1) + 1 + OUT_PAD  # 128
OWW = OHH  # 128

USE_CUSTOM_DVE = os.environ.get("DCT_CUSTOM_DVE", "0") == "1"
STUB = set(os.environ.get("DCT_STUB", "").split(","))  # nogather, smallgather, nooffset

NBUCK = OHH + 1        # buckets bi in [0, 128]
CAP = 384              # bucket capacity (max observed fill 338)
GB = int(os.environ.get("DCT_GB", "1"))  # buckets per gather group
NGRP = (NBUCK + GB - 1) // GB  # 33 (last group has 1 bucket)
NPIX = HW              # 4096
NPT = NPIX // 128      # 32 pixel tiles
VROWS = NPIX * NK      # 36864


# ---------------------------------------------------------------------------
# Custom DVE op: out = relu(1 - |in0 - s0|) * s1
# ---------------------------------------------------------------------------
_HATWY = None


def _get_hatwy_op():
    global _HATWY
    if _HATWY is not None:
        return _HATWY
    from concourse.dve_spec import Spec, Src0, C0, C1, Zero, One, maxx, relu
    from concourse import dve_ops
    from concourse.dve_ops import DveOp, OPS

    d = Src0 - C0
    a = maxx(d, Zero - d)
    spec = Spec(
        body=relu(One - a) * C1,
        reference=lambda in0, in1, s0, s1, imm2: (
            np.maximum(1.0 - np.abs(in0 - s0), 0.0) * s1
        ),
    )
    op = DveOp("HATWY_DCT_ANT", spec, subdim=False, uops_sha={})
    OPS.append(op)
    dve_ops.CUSTOM_DVE_SPECS[op.name] = op.spec
    dve_ops._SUB_OPCODE_FOR_NAME[op.name] = (
        dve_ops._CUSTOM_DVE_ROW_BASE + len(OPS) - 1)
    assert dve_ops._SUB_OPCODE_FOR_NAME[op.name] < 0x20
    for ver in ("v3", "v4"):
        try:
            op.compile(ver)
        except ValueError as e:
            m = re.search(r"drifted \(v\d: ([0-9a-f]+)", str(e))
            if not m:
                raise
            op.uops_sha[ver] = m.group(1)
            op.compile(ver)
    _HATWY = op
    return op


def _split_multi_waits(nc, max_waits=1):
    """Walrus accepts one sync wait per instruction; hoist extras."""
    n = 0
    for fn in nc.m.functions:
        for bb in fn.blocks:
            out = []
            changed = False
            for inst in bb.instructions:
                si = inst.sync_info
                if si is not None and si.on_wait and len(si.on_wait) > max_waits:
                    waits = list(si.on_wait)
                    for w in waits[:-max_waits]:
                        ev = mybir.InstEventSemaphore(
                            name=f"evsplit-{n}",
                            engine=inst.engine,
                            ins=[],
                            outs=[],
                            sync_info=mybir.SyncInfo(on_wait=[w], on_update=[]),
                        )
                        n += 1
                        nc.register_instruction(ev, overwrite=True)
                        out.append(ev)
                    inst.sync_info = mybir.SyncInfo(
                        on_wait=waits[-max_waits:],
                        on_update=list(si.on_update or []),
                    )
                    changed = True
                out.append(inst)
            if changed:
                bb.instructions = out


# ---------------------------------------------------------------------------
# Host-side preprocessing
# ---------------------------------------------------------------------------
def _geometry(offset_b):
    off = offset_b.reshape(NK, 2, H, W).astype(np.float64)
    ky = (np.arange(NK) // KW).astype(np.float64) * DIL
    kx = (np.arange(NK) % KW).astype(np.float64) * DIL
    base_y = np.arange(H, dtype=np.float64) * STRIDE - PAD
    base_x = np.arange(W, dtype=np.float64) * STRIDE - PAD
    y = (off[:, 0] + base_y[None, :, None] + ky[:, None, None]).reshape(NK, HW)
    xp = (off[:, 1] + base_x[None, None, :] + kx[:, None, None]).reshape(NK, HW)
    return y, xp


def _prep_windows(offset):
    """Shared (all-core) gather window bases per group + pass-A readiness."""
    lo = np.full(NGRP, VROWS, np.int64)
    hi = np.zeros(NGRP, np.int64)
    for b in range(B):
        y, _ = _geometry(offset[b])
        bi = (np.floor(y) + 1).astype(np.int64)
        flat = (np.arange(HW)[None, :] * NK + np.arange(NK)[:, None])
        valid = (bi >= 0) & (bi <= OHH)
        g = bi // GB
        for gi in range(NGRP):
            m = valid & (g == gi)
            if m.any():
                lo[gi] = min(lo[gi], flat[m].min())
                hi[gi] = max(hi[gi], flat[m].max())
    lo = np.minimum(lo, hi)  # empty-group safety
    span = hi - lo + 1
    assert span.max() <= 32768, f"window span {span.max()} exceeds int16 range"
    # pass-A pixel tile needed before group gi can gather (V rows < (pt+1)*1152)
    pt_needed = np.minimum((hi // (128 * NK)).astype(np.int64), NPT - 1)
    return lo.astype(np.int64), span.astype(np.int64), pt_needed


def _prep_core(x_b, offset_b, mask_b, wlo):
    y, xp = _geometry(offset_b)
    m = mask_b.reshape(NK, HW).astype(np.float64)

    y0 = np.floor(y)
    fy = (y - y0).reshape(-1)
    bi = (y0 + 1).astype(np.int64).reshape(-1)
    flat = (np.arange(HW)[None, :] * NK + np.arange(NK)[:, None]).reshape(-1)
    xpf = xp.reshape(-1)
    mf = m.reshape(-1)

    valid = (bi >= 0) & (bi <= OHH)
    vb = bi[valid]
    vf = flat[valid]
    order = np.lexsort((vf, vb))
    vb = vb[order]
    vf = vf[order]
    vxp = xpf[valid][order]
    vfy = fy[valid][order]
    vm = mf[valid][order]

    start = np.searchsorted(vb, np.arange(NBUCK), side="left")
    counts = np.searchsorted(vb, np.arange(NBUCK), side="right") - start
    if counts.max() > CAP:
        raise RuntimeError(f"bucket overflow: {counts.max()} > {CAP}")
    rank = np.arange(len(vb)) - start[vb]
    slot = vb * CAP + rank  # global slot id

    gx = np.zeros(NBUCK * CAP, np.int16)
    xpa = np.full(NBUCK * CAP, 3000.0, np.float32)
    wy0 = np.zeros(NBUCK * CAP, np.float32)
    wy1 = np.zeros(NBUCK * CAP, np.float32)

    rel = vf - wlo[vb // GB]
    assert rel.min() >= 0 and rel.max() < 32768
    gx[slot] = rel.astype(np.int16)
    xpa[slot] = vxp
    w0 = (1.0 - vfy) * vm
    w1 = vfy * vm
    w0[vb == 0] = 0.0
    w1[vb == OHH] = 0.0
    if not USE_CUSTOM_DVE:
        # stock path computes t = min(|d|-1, 0) and multiplies by -wy
        w0 = -w0
        w1 = -w1
    wy0[slot] = w0
    wy1[slot] = w1

    # gather idx: per-bucket 16-wrap, replicated to 128 partitions
    w16 = gx.reshape(NBUCK, CAP // 16, 16).transpose(2, 0, 1)  # [16, NBUCK, 24]
    gxt = np.ascontiguousarray(np.tile(w16, (8, 1, 1)))        # [128, NBUCK, 24]

    def slotmaj(a):
        # [NBUCK*CAP] -> [128 p, NBUCK, 3 c, 1]
        return np.ascontiguousarray(
            a.reshape(NBUCK, 3, 128).transpose(2, 0, 1))[..., None]

    xs = np.zeros((128, 2, NPIX), BF16)
    xr = x_b.reshape(2, 128, NPIX)  # cin-half, cin-in-half, pixel
    xs[:, 0, :] = xr[0]
    xs[:, 1, :] = xr[1]

    return {
        "xs": xs,
        "gx": gxt,
        "xpa": slotmaj(xpa),
        "wy0": slotmaj(wy0),
        "wy1": slotmaj(wy1),
    }


def _prep_all(x, weight, offset, mask, bias):
    wlo, wspan, pt_needed = _prep_windows(offset)
    wd = np.ascontiguousarray(
        weight.reshape(CIN, COUT, NK).transpose(0, 2, 1).reshape(2, 128, NK * COUT)
    ).astype(BF16)
    bv = bias.reshape(1, COUT).astype(BF16)
    in_maps = []
    for b in range(B):
        d = _prep_core(x[b].reshape(CIN, HW).astype(BF16), offset[b], mask[b],
                       wlo)
        d["wd"] = wd
        d["bias"] = bv
        in_maps.append(d)
    return in_maps, (tuple(wlo), tuple(wspan), tuple(pt_needed))


# ---------------------------------------------------------------------------
# Device program
# ---------------------------------------------------------------------------
def build_nc(params, reps=1):
    wlo, wspan, pt_needed = params
    nc = bass.Bass()
    i16, i32 = mybir.dt.int16, mybir.dt.int32
    bf, f32 = mybir.dt.bfloat16, mybir.dt.float32
    hatop = _get_hatwy_op() if USE_CUSTOM_DVE else None
    AL = mybir.AluOpType

    xsd = nc.dram_tensor("xs", [128, 2, NPIX], bf, kind="ExternalInput")
    gxd = nc.dram_tensor("gx", [128, NBUCK, CAP // 16], i16,
                         kind="ExternalInput")
    xpad = nc.dram_tensor("xpa", [128, NBUCK, 3, 1], f32,
                          kind="ExternalInput")
    wy0d = nc.dram_tensor("wy0", [128, NBUCK, 3, 1], f32,
                          kind="ExternalInput")
    wy1d = nc.dram_tensor("wy1", [128, NBUCK, 3, 1], f32,
                          kind="ExternalInput")
    wdd = nc.dram_tensor("wd", [2, 128, NK * COUT], bf, kind="ExternalInput")
    bd = nc.dram_tensor("bias", [1, COUT], bf, kind="ExternalInput")
    outd = nc.dram_tensor("out", [OHH, OWW, COUT], bf, kind="ExternalOutput")

    with tile.TileContext(nc) as tc:
        with tc.tile_pool(name="const", bufs=1) as cpool, \
             tc.tile_pool(name="slab", bufs=3) as slabpool, \
             tc.tile_pool(name="vd", bufs=1, space="DRAM") as vdpool, \
             tc.tile_pool(name="vg", bufs=3) as vgpool, \
             tc.tile_pool(name="hat", bufs=14) as hatpool, \
             tc.tile_pool(name="ob", bufs=6) as opool, \
             tc.tile_pool(name="pg", bufs=3, space="PSUM") as pgpool, \
             tc.tile_pool(name="po", bufs=5, space="PSUM") as popool:

            xst = cpool.tile([128, 2, NPIX], bf, tag="xs")
            nc.sync.dma_start(out=xst[:], in_=xsd[:])
            wt = []
            for h in range(2):
                t = cpool.tile([128, NK * COUT], bf, tag=f"w{h}")
                nc.sync.dma_start(out=t[:], in_=wdd[h])
                wt.append(t)
            gxt = cpool.tile([128, NBUCK, CAP // 16], i16, tag="gx")
            nc.sync.dma_start(out=gxt[:], in_=gxd[:])
            xpat = cpool.tile([128, NBUCK, 3, 1], f32, tag="xpa")
            nc.sync.dma_start(out=xpat[:], in_=xpad[:])
            wyts = []
            for name, dram in (("wy0", wy0d), ("wy1", wy1d)):
                t = cpool.tile([128, NBUCK, 3, 1], f32, tag=name)
                nc.sync.dma_start(out=t[:], in_=dram[:])
                wyts.append(t)
            biast = cpool.tile([1, COUT], bf, tag="bias")
            nc.sync.dma_start(out=biast[:], in_=bd[:])
            onest = cpool.tile([1, OWW], bf, tag="ones")
            nc.vector.memset(onest[:], 1.0)
            ioI = cpool.tile([128, OWW], i32, tag="ioI")
            nc.gpsimd.iota(ioI[:], pattern=[[1, OWW]], base=0,
                           channel_multiplier=0)
            nc.gpsimd.load_library(library_config.mlp)
            iotaB = cpool.tile([128, OWW], bf, tag="iotaB")
            nc.vector.tensor_copy(out=iotaB[:], in_=ioI[:])
            ioF1 = cpool.tile([128, OWW], f32, tag="ioF1")
            nc.vector.tensor_copy(out=ioF1[:], in_=ioI[:])
            nidx_regs = {n: nc.gpsimd.to_reg(n)
                         for n in {GB * CAP, (NBUCK - (NGRP - 1) * GB) * CAP,
                                   CAP}}

            vd = vdpool.tile([VROWS, COUT], bf, tag="vd")

            for rep in range(reps):
                # PSUM row tiles keyed by output row
                rowps = {}

                def open_row(r):
                    ps = popool.tile([OWW, COUT], f32, tag="po")
                    rowps[r] = ps
                    return ps

                def emit_row(r):
                    ps = rowps.pop(r)
                    nc.tensor.matmul(
                        out=ps[:], lhsT=onest[0:1, :], rhs=biast[0:1, :],
                        start=False, stop=True)
                    ob = opool.tile([OWW, COUT], bf, tag="ob")
                    nc.scalar.copy(out=ob[:], in_=ps[:])
                    nc.scalar.dma_start(out=outd[r], in_=ob[:])

                def pass_a(pt):
                    slab = slabpool.tile([128, NK * COUT], bf, tag="slab")
                    for tp in range(5):
                        wlen = 512 if tp < 4 else 256
                        pg = pgpool.tile([128, 512], f32, tag="pg")
                        for h in range(2):
                            nc.tensor.matmul(
                                out=pg[:, :wlen],
                                lhsT=xst[:, h, 128 * pt:128 * (pt + 1)],
                                rhs=wt[h][:, 512 * tp:512 * tp + wlen],
                                start=(h == 0), stop=(h == 1))
                        nc.scalar.copy(
                            out=slab[:, 512 * tp:512 * tp + wlen],
                            in_=pg[:, :wlen])
                    nc.sync.dma_start(
                        out=vd[1152 * pt:1152 * (pt + 1), :], in_=slab[:])

                def pass_b(g):
                    blo = GB * g
                    bhi = min(blo + GB, NBUCK)
                    ntb = bhi - blo
                    nid = ntb * CAP
                    vg = vgpool.tile([128, GB * 3, COUT], bf, tag="vg")
                    if "nogather" in STUB:
                        nc.vector.memset(vg[:, :ntb * 3, :], 0.25)
                    elif "smallgather" in STUB:
                        for bi_ in range(blo, bhi):
                            off = 0 if "nooffset" in STUB else wlo[g]
                            span = 32768 if "nooffset" in STUB else wspan[g]
                            nc.gpsimd.dma_gather(
                                out_ap=vg[:, 3 * (bi_ - blo):
                                          3 * (bi_ - blo) + 3, :],
                                in_ap=vd[off:off + span, :],
                                idxs_ap=gxt[:, bi_:bi_ + 1, :],
                                num_idxs=CAP,
                                num_idxs_reg=nidx_regs[CAP],
                                elem_size=COUT,
                            )
                    else:
                        off = 0 if "nooffset" in STUB else wlo[g]
                        span = 32768 if "nooffset" in STUB else wspan[g]
                        nc.gpsimd.dma_gather(
                            out_ap=vg[:, :ntb * 3, :],
                            in_ap=vd[off:off + span, :],
                            idxs_ap=gxt[:, blo:bhi, :],
                            num_idxs=nid,
                            num_idxs_reg=nidx_regs[nid],
                            elem_size=COUT,
                        )
                    for bi in range(blo, bhi):
                        cb = 3 * (bi - blo)
                        hws = [None, None]
                        if not USE_CUSTOM_DVE:
                            # stock-op hat build, batched over the 3 c-tiles
                            tmp = hatpool.tile([128, 3, OWW], bf, tag="tmp")
                            nc.vector.tensor_tensor(
                                out=tmp[:],
                                in0=ioF1[:, None, :].to_broadcast(
                                    [128, 3, OWW]),
                                in1=xpat[:, bi].to_broadcast([128, 3, OWW]),
                                op=AL.subtract)
                            tmpi = tmp[:].bitcast(i16)
                            nc.vector.tensor_scalar(
                                out=tmpi, in0=tmpi, scalar1=0x7FFF,
                                scalar2=None, op0=AL.bitwise_and)
                            nc.vector.tensor_scalar(
                                out=tmp[:], in0=tmp[:], scalar1=1.0,
                                op0=AL.subtract, scalar2=0.0, op1=AL.min)
                            for corner in range(2):
                                hw = hatpool.tile([128, 3, OWW], bf,
                                                  tag="hw")
                                nc.vector.tensor_tensor(
                                    out=hw[:], in0=tmp[:],
                                    in1=wyts[corner][:, bi].to_broadcast(
                                        [128, 3, OWW]),
                                    op=AL.mult)
                                hws[corner] = hw

                        def lhs(corner, c):
                            if USE_CUSTOM_DVE:
                                hat = hatpool.tile([128, OWW], bf, tag="hat")
                                nc.vector._custom_dve(
                                    hatop, out=hat[:], in0=iotaB[:],
                                    s0=xpat[:, bi, c],
                                    s1=wyts[corner][:, bi, c])
                                return hat[:]
                            return hws[corner][:, c, :]

                        # corner dy=0 -> row bi-1 (closes it)
                        if bi >= 1:
                            ps = rowps[bi - 1]
                            for c in range(3):
                                nc.tensor.matmul(
                                    out=ps[:], lhsT=lhs(0, c),
                                    rhs=vg[:, cb + c, :],
                                    start=False, stop=False)
                            emit_row(bi - 1)
                        # corner dy=1 -> row bi (opens it)
                        if bi <= OHH - 1:
                            ps = open_row(bi)
                            for c in range(3):
                                nc.tensor.matmul(
                                    out=ps[:], lhsT=lhs(1, c),
                                    rhs=vg[:, cb + c, :],
                                    start=(c == 0), stop=False)

                # interleaved schedule: pass_a(pt) then ready groups
                gnext = 0
                for pt in range(NPT):
                    pass_a(pt)
                    while gnext < NGRP and pt_needed[gnext] <= pt:
                        pass_b(gnext)
                        gnext += 1
                for g in range(gnext, NGRP):
                    pass_b(g)
                assert not rowps, f"unclosed rows {list(rowps)}"

    lower_extended_insts(nc)
    _split_multi_waits(nc)
    return nc


# ---------------------------------------------------------------------------
# Runner (compile/load once; dispatch cheaply)
# ---------------------------------------------------------------------------
class Runner:
    def __init__(self, params, reps=1):
        import jax
        import jax.numpy as jnp
        from jax.sharding import Mesh, PartitionSpec
        from jax.experimental.shard_map import shard_map
        from concourse.bass2jax import (
            _bass_exec_p, install_neuronx_cc_hook, partition_id_tensor,
        )

        install_neuronx_cc_hook()
        nc = build_nc(params, reps)
        self.nc = nc
        in_names, out_names, out_avals = [], [], []
        pname = nc.partition_id_tensor.name if nc.partition_id_tensor else None
        for alloc in nc.m.functions[0].allocations:
            if not isinstance(alloc, mybir.MemoryLocationSet):
                continue
            name = alloc.memorylocations[0].name
            if alloc.kind == "ExternalInput":
                if name != pname:
                    in_names.append(name)
            elif alloc.kind == "ExternalOutput":
                shape = tuple(alloc.tensor_shape)
                dtype = mybir.dt.np(alloc.dtype)
                out_avals.append(jax.core.ShapedArray(shape, dtype))
                out_names.append(name)
        self.in_names, self.out_names = in_names, out_names
        self.out_avals = out_avals
        n_params = len(in_names)
        all_in = in_names + out_names + ([pname] if pname else [])

        def _body(*args):
            operands = list(args)
            if pname:
                operands.append(partition_id_tensor())
            return tuple(_bass_exec_p.bind(
                *operands, out_avals=tuple(out_avals), in_names=tuple(all_in),
                out_names=tuple(out_names), lowering_input_output_aliases=(),
                sim_require_finite=True, sim_require_nnan=True, nc=nc))

        devices = jax.devices()[:B]
        mesh = Mesh(np.asarray(devices), ("core",))
        in_specs = (PartitionSpec("core"),) * (n_params + len(out_avals))
        out_specs = (PartitionSpec("core"),) * len(out_names)
        self._mesh = mesh
        self._shard_body = shard_map(
            _body, mesh=mesh, in_specs=in_specs, out_specs=out_specs,
            check_rep=False,
        )
        donate = tuple(range(n_params, n_params + len(out_avals)))
        self._jit = jax.jit(self._shard_body, donate_argnums=donate,
                            keep_unused=True)
        self._jax = jax
        from jax.sharding import NamedSharding
        sh = NamedSharding(mesh, PartitionSpec("core"))
        zshapes = [((B * av.shape[0], *av.shape[1:]), av.dtype)
                   for av in out_avals]

        def _mk_zeros():
            return tuple(jnp.zeros(s, d) for s, d in zshapes)

        self._mk_zeros = jax.jit(_mk_zeros, out_shardings=(sh,) * len(zshapes))

    def concat_inputs(self, in_maps):
        return [np.concatenate([np.asarray(m[n]) for m in in_maps], axis=0)
                for n in self.in_names]

    def __call__(self, concat_in):
        outs = self._jit(*concat_in, *self._mk_zeros())
        self._jax.block_until_ready(outs)
        return [
            {name: np.asarray(outs[i]).reshape(B, *self.out_avals[i].shape)[c]
             for i, name in enumerate(self.out_names)}
            for c in range(B)
        ]

    def make_timing_fn(self, concat_in):
        """Device-resident operands: warm calls measure dispatch + exec."""
        import jax
        from jax.sharding import NamedSharding, PartitionSpec

        sh = NamedSharding(self._mesh, PartitionSpec("core"))
        dev_args = [jax.device_put(a, sh) for a in concat_in]
        jf = self._jit
        jax.block_until_ready(jf(*dev_args, *self._mk_zeros()))

        def call():
            outs = jf(*dev_args, *self._mk_zeros())
            jax.block_until_ready(outs)
            return outs
        return call


_RUNNERS = {}


def get_runner(reps=1, params=None):
    key = (reps, params)
    if key not in _RUNNERS:
        _RUNNERS[key] = Runner(params, reps)
    return _RUNNERS[key]


_LAST_PARAMS = None


def kernel(x, weight, offset, mask, bias):
    global _LAST_PARAMS
    x = np.asarray(x, dtype=np.float32)
    weight = np.asarray(weight, dtype=np.float32)
    offset = np.asarray(offset, dtype=np.float32)
    mask = np.asarray(mask, dtype=np.float32)
    bias = np.asarray(bias, dtype=np.float32)

    in_maps, params = _prep_all(x, weight, offset, mask, bias)
    _LAST_PARAMS = params
    r = get_runner(1, params)
    results = r(r.concat_inputs(in_maps))
    out = np.empty((B, COUT, OHH, OWW), dtype=np.float32)
    for b in range(B):
        od = results[b]["out"].astype(np.float32)  # [oy, ox, cout]
        out[b] = od.transpose(2, 0, 1)
    return out


# revision 22
# speedup vs baseline: 2.4902x; 2.4902x over previous
"""DeformConvTranspose2d Bass kernel for 8 Trainium2 NeuronCores — v3.

Data-parallel over batch (one batch element per core).

Per core:
  Pass A (dense GEMM): V[pixel, tap, cout] = x_col @ W, computed on 32
  pixel-tiles of 128 (no bucket padding), written to DRAM VD [36864, 256].
  Pass B (scatter): entries (pixel, tap) bucketed by output row index
  bi = floor(y)+1 (mixed taps, capacity 384 = 3 K-tiles).  dma_gather
  (transpose=False, 4 buckets per call, per-group window base keeps the
  int16 indices small) pulls V rows slot-on-partition.  A custom fused
  DVE op builds lhsT = relu(1 - |iota - xpa|) * wy per (tile, corner);
  out_psum[ox 128, cout 256] accumulates 3 matmuls from bucket r
  (corner dy=1) + 3 from bucket r+1 (corner dy=0) + rank-1 bias matmul,
  then ACT copies to bf16 and DMA writes out[r] of [oy, ox, cout].

Host: prep bucketization (vectorized numpy), final transpose to
[cout, oy, ox] f32.
"""

import os
import re
import sys

sys.path.insert(0, "/opt/trn_rl_repo")

import numpy as np

from concourse import bass, mybir, library_config
from concourse.library_overlay import lower_extended_insts
import concourse.tile as tile

BF16 = mybir.dt.np(mybir.dt.bfloat16)

# problem constants (hardcoded per contract)
B = 8
CIN = 256
COUT = 256
H = W = 64
HW = H * W
NK = 9
KH = KW = 3
STRIDE, PAD, OUT_PAD, DIL = 2, 1, 1, 1
OHH = (H - 1) * STRIDE - 2 * PAD + DIL * (KH - 1) + 1 + OUT_PAD  # 128
OWW = OHH  # 128

USE_CUSTOM_DVE = os.environ.get("DCT_CUSTOM_DVE", "0") == "1"
STUB = set(os.environ.get("DCT_STUB", "").split(","))  # nogather, smallgather, nooffset

NBUCK = OHH + 1        # buckets bi in [0, 128]
CAP = 384              # bucket capacity (max observed fill 338)
GB = int(os.environ.get("DCT_GB", "1"))  # buckets per gather group
NGRP = (NBUCK + GB - 1) // GB  # 33 (last group has 1 bucket)
NPIX = HW              # 4096
NPT = NPIX // 128      # 32 pixel tiles
VROWS = NPIX * NK      # 36864


# ---------------------------------------------------------------------------
# Custom DVE op: out = relu(1 - |in0 - s0|) * s1
# ---------------------------------------------------------------------------
_HATWY = None


def _get_hatwy_op():
    global _HATWY
    if _HATWY is not None:
        return _HATWY
    from concourse.dve_spec import Spec, Src0, C0, C1, Zero, One, maxx, relu
    from concourse import dve_ops
    from concourse.dve_ops import DveOp, OPS

    d = Src0 - C0
    a = maxx(d, Zero - d)
    spec = Spec(
        body=relu(One - a) * C1,
        reference=lambda in0, in1, s0, s1, imm2: (
            np.maximum(1.0 - np.abs(in0 - s0), 0.0) * s1
        ),
    )
    op = DveOp("HATWY_DCT_ANT", spec, subdim=False, uops_sha={})
    OPS.append(op)
    dve_ops.CUSTOM_DVE_SPECS[op.name] = op.spec
    dve_ops._SUB_OPCODE_FOR_NAME[op.name] = (
        dve_ops._CUSTOM_DVE_ROW_BASE + len(OPS) - 1)
    assert dve_ops._SUB_OPCODE_FOR_NAME[op.name] < 0x20
    for ver in ("v3", "v4"):
        try:
            op.compile(ver)
        except ValueError as e:
            m = re.search(r"drifted \(v\d: ([0-9a-f]+)", str(e))
            if not m:
                raise
            op.uops_sha[ver] = m.group(1)
            op.compile(ver)
    _HATWY = op
    return op


def _split_multi_waits(nc, max_waits=1):
    """Walrus accepts one sync wait per instruction; hoist extras."""
    n = 0
    for fn in nc.m.functions:
        for bb in fn.blocks:
            out = []
            changed = False
            for inst in bb.instructions:
                si = inst.sync_info
                if si is not None and si.on_wait and len(si.on_wait) > max_waits:
                    waits = list(si.on_wait)
                    for w in waits[:-max_waits]:
                        ev = mybir.InstEventSemaphore(
                            name=f"evsplit-{n}",
                            engine=inst.engine,
                            ins=[],
                            outs=[],
                            sync_info=mybir.SyncInfo(on_wait=[w], on_update=[]),
                        )
                        n += 1
                        nc.register_instruction(ev, overwrite=True)
                        out.append(ev)
                    inst.sync_info = mybir.SyncInfo(
                        on_wait=waits[-max_waits:],
                        on_update=list(si.on_update or []),
                    )
                    changed = True
                out.append(inst)
            if changed:
                bb.instructions = out


# ---------------------------------------------------------------------------
# Host-side preprocessing
# ---------------------------------------------------------------------------
def _geometry(offset_b):
    off = offset_b.reshape(NK, 2, H, W).astype(np.float64)
    ky = (np.arange(NK) // KW).astype(np.float64) * DIL
    kx = (np.arange(NK) % KW).astype(np.float64) * DIL
    base_y = np.arange(H, dtype=np.float64) * STRIDE - PAD
    base_x = np.arange(W, dtype=np.float64) * STRIDE - PAD
    y = (off[:, 0] + base_y[None, :, None] + ky[:, None, None]).reshape(NK, HW)
    xp = (off[:, 1] + base_x[None, None, :] + kx[:, None, None]).reshape(NK, HW)
    return y, xp


def _prep_windows(offset):
    """Shared (all-core) gather window bases per group + pass-A readiness."""
    lo = np.full(NGRP, VROWS, np.int64)
    hi = np.zeros(NGRP, np.int64)
    for b in range(B):
        y, _ = _geometry(offset[b])
        bi = (np.floor(y) + 1).astype(np.int64)
        flat = (np.arange(HW)[None, :] * NK + np.arange(NK)[:, None])
        valid = (bi >= 0) & (bi <= OHH)
        g = bi // GB
        for gi in range(NGRP):
            m = valid & (g == gi)
            if m.any():
                lo[gi] = min(lo[gi], flat[m].min())
                hi[gi] = max(hi[gi], flat[m].max())
    lo = np.minimum(lo, hi)  # empty-group safety
    span = hi - lo + 1
    assert span.max() <= 32768, f"window span {span.max()} exceeds int16 range"
    # pass-A pixel tile needed before group gi can gather (V rows < (pt+1)*1152)
    pt_needed = np.minimum((hi // (128 * NK)).astype(np.int64), NPT - 1)
    return lo.astype(np.int64), span.astype(np.int64), pt_needed


def _prep_core(x_b, offset_b, mask_b, wlo):
    y, xp = _geometry(offset_b)
    m = mask_b.reshape(NK, HW).astype(np.float64)

    y0 = np.floor(y)
    fy = (y - y0).reshape(-1)
    bi = (y0 + 1).astype(np.int64).reshape(-1)
    flat = (np.arange(HW)[None, :] * NK + np.arange(NK)[:, None]).reshape(-1)
    xpf = xp.reshape(-1)
    mf = m.reshape(-1)

    valid = (bi >= 0) & (bi <= OHH)
    vb = bi[valid]
    vf = flat[valid]
    order = np.lexsort((vf, vb))
    vb = vb[order]
    vf = vf[order]
    vxp = xpf[valid][order]
    vfy = fy[valid][order]
    vm = mf[valid][order]

    start = np.searchsorted(vb, np.arange(NBUCK), side="left")
    counts = np.searchsorted(vb, np.arange(NBUCK), side="right") - start
    if counts.max() > CAP:
        raise RuntimeError(f"bucket overflow: {counts.max()} > {CAP}")
    rank = np.arange(len(vb)) - start[vb]
    slot = vb * CAP + rank  # global slot id

    gx = np.zeros(NBUCK * CAP, np.int16)
    xpa = np.full(NBUCK * CAP, 3000.0, np.float32)
    wy0 = np.zeros(NBUCK * CAP, np.float32)
    wy1 = np.zeros(NBUCK * CAP, np.float32)

    rel = vf - wlo[vb // GB]
    assert rel.min() >= 0 and rel.max() < 32768
    gx[slot] = rel.astype(np.int16)
    xpa[slot] = vxp
    w0 = (1.0 - vfy) * vm
    w1 = vfy * vm
    w0[vb == 0] = 0.0
    w1[vb == OHH] = 0.0
    if not USE_CUSTOM_DVE:
        # stock path computes t = min(|d|-1, 0) and multiplies by -wy
        w0 = -w0
        w1 = -w1
    wy0[slot] = w0
    wy1[slot] = w1

    # gather idx: per-bucket 16-wrap, replicated to 128 partitions
    w16 = gx.reshape(NBUCK, CAP // 16, 16).transpose(2, 0, 1)  # [16, NBUCK, 24]
    gxt = np.ascontiguousarray(np.tile(w16, (8, 1, 1)))        # [128, NBUCK, 24]

    def slotmaj(a):
        # [NBUCK*CAP] -> [128 p, NBUCK, 3 c, 1]
        return np.ascontiguousarray(
            a.reshape(NBUCK, 3, 128).transpose(2, 0, 1))[..., None]

    xs = np.zeros((128, 2, NPIX), BF16)
    xr = x_b.reshape(2, 128, NPIX)  # cin-half, cin-in-half, pixel
    xs[:, 0, :] = xr[0]
    xs[:, 1, :] = xr[1]

    return {
        "xs": xs,
        "gx": gxt,
        "xpa": slotmaj(xpa),
        "wy0": slotmaj(wy0),
        "wy1": slotmaj(wy1),
    }


def _prep_all(x, weight, offset, mask, bias):
    wlo, wspan, pt_needed = _prep_windows(offset)
    wd = np.ascontiguousarray(
        weight.reshape(CIN, COUT, NK).transpose(0, 2, 1).reshape(2, 128, NK * COUT)
    ).astype(BF16)
    bv = bias.reshape(1, COUT).astype(BF16)
    in_maps = []
    for b in range(B):
        d = _prep_core(x[b].reshape(CIN, HW).astype(BF16), offset[b], mask[b],
                       wlo)
        d["wd"] = wd
        d["bias"] = bv
        in_maps.append(d)
    return in_maps, (tuple(wlo), tuple(wspan), tuple(pt_needed))


# ---------------------------------------------------------------------------
# Device program
# ---------------------------------------------------------------------------
def build_nc(params, reps=1):
    wlo, wspan, pt_needed = params
    nc = bass.Bass()
    i16, i32 = mybir.dt.int16, mybir.dt.int32
    bf, f32 = mybir.dt.bfloat16, mybir.dt.float32
    hatop = _get_hatwy_op() if USE_CUSTOM_DVE else None
    AL = mybir.AluOpType

    xsd = nc.dram_tensor("xs", [128, 2, NPIX], bf, kind="ExternalInput")
    gxd = nc.dram_tensor("gx", [128, NBUCK, CAP // 16], i16,
                         kind="ExternalInput")
    xpad = nc.dram_tensor("xpa", [128, NBUCK, 3, 1], f32,
                          kind="ExternalInput")
    wy0d = nc.dram_tensor("wy0", [128, NBUCK, 3, 1], f32,
                          kind="ExternalInput")
    wy1d = nc.dram_tensor("wy1", [128, NBUCK, 3, 1], f32,
                          kind="ExternalInput")
    wdd = nc.dram_tensor("wd", [2, 128, NK * COUT], bf, kind="ExternalInput")
    bd = nc.dram_tensor("bias", [1, COUT], bf, kind="ExternalInput")
    outd = nc.dram_tensor("out", [OHH, OWW, COUT], bf, kind="ExternalOutput")

    with tile.TileContext(nc) as tc:
        with tc.tile_pool(name="const", bufs=1) as cpool, \
             tc.tile_pool(name="slab", bufs=3) as slabpool, \
             tc.tile_pool(name="vd", bufs=1, space="DRAM") as vdpool, \
             tc.tile_pool(name="vg", bufs=4) as vgpool, \
             tc.tile_pool(name="hat", bufs=14) as hatpool, \
             tc.tile_pool(name="ob", bufs=6) as opool, \
             tc.tile_pool(name="pg", bufs=3, space="PSUM") as pgpool, \
             tc.tile_pool(name="po", bufs=5, space="PSUM") as popool:

            xst = cpool.tile([128, 2, NPIX], bf, tag="xs")
            nc.sync.dma_start(out=xst[:], in_=xsd[:])
            wt = []
            for h in range(2):
                t = cpool.tile([128, NK * COUT], bf, tag=f"w{h}")
                nc.sync.dma_start(out=t[:], in_=wdd[h])
                wt.append(t)
            gxt = cpool.tile([128, NBUCK, CAP // 16], i16, tag="gx")
            nc.sync.dma_start(out=gxt[:], in_=gxd[:])
            xpat = cpool.tile([128, NBUCK, 3, 1], f32, tag="xpa")
            nc.sync.dma_start(out=xpat[:], in_=xpad[:])
            wyts = []
            for name, dram in (("wy0", wy0d), ("wy1", wy1d)):
                t = cpool.tile([128, NBUCK, 3, 1], f32, tag=name)
                nc.sync.dma_start(out=t[:], in_=dram[:])
                wyts.append(t)
            biast = cpool.tile([1, COUT], bf, tag="bias")
            nc.sync.dma_start(out=biast[:], in_=bd[:])
            onest = cpool.tile([1, OWW], bf, tag="ones")
            nc.vector.memset(onest[:], 1.0)
            ioI = cpool.tile([128, OWW], i32, tag="ioI")
            nc.gpsimd.iota(ioI[:], pattern=[[1, OWW]], base=0,
                           channel_multiplier=0)
            nc.gpsimd.load_library(library_config.mlp)
            iotaB = cpool.tile([128, OWW], bf, tag="iotaB")
            nc.vector.tensor_copy(out=iotaB[:], in_=ioI[:])
            ioF1 = cpool.tile([128, OWW], f32, tag="ioF1")
            nc.vector.tensor_copy(out=ioF1[:], in_=ioI[:])
            nidx_regs = {n: nc.gpsimd.to_reg(n)
                         for n in {GB * CAP, (NBUCK - (NGRP - 1) * GB) * CAP,
                                   CAP}}

            vd = vdpool.tile([VROWS, COUT], bf, tag="vd")

            for rep in range(reps):
                # PSUM row tiles keyed by output row
                rowps = {}

                def open_row(r):
                    ps = popool.tile([OWW, COUT], f32, tag="po")
                    rowps[r] = ps
                    return ps

                def emit_row(r):
                    ps = rowps.pop(r)
                    nc.tensor.matmul(
                        out=ps[:], lhsT=onest[0:1, :], rhs=biast[0:1, :],
                        start=False, stop=True)
                    ob = opool.tile([OWW, COUT], bf, tag="ob")
                    nc.scalar.copy(out=ob[:], in_=ps[:])
                    nc.scalar.dma_start(out=outd[r], in_=ob[:])

                def pass_a(pt):
                    slab = slabpool.tile([128, NK * COUT], bf, tag="slab")
                    for tp in range(5):
                        wlen = 512 if tp < 4 else 256
                        pg = pgpool.tile([128, 512], f32, tag="pg")
                        for h in range(2):
                            nc.tensor.matmul(
                                out=pg[:, :wlen],
                                lhsT=xst[:, h, 128 * pt:128 * (pt + 1)],
                                rhs=wt[h][:, 512 * tp:512 * tp + wlen],
                                start=(h == 0), stop=(h == 1))
                        nc.scalar.copy(
                            out=slab[:, 512 * tp:512 * tp + wlen],
                            in_=pg[:, :wlen])
                    nc.sync.dma_start(
                        out=vd[1152 * pt:1152 * (pt + 1), :], in_=slab[:])

                def pass_b(g):
                    blo = GB * g
                    bhi = min(blo + GB, NBUCK)
                    ntb = bhi - blo
                    nid = ntb * CAP
                    vg = vgpool.tile([128, GB * 3, COUT], bf, tag="vg")
                    if "nogather" in STUB:
                        nc.vector.memset(vg[:, :ntb * 3, :], 0.25)
                    elif "smallgather" in STUB:
                        for bi_ in range(blo, bhi):
                            off = 0 if "nooffset" in STUB else wlo[g]
                            span = 32768 if "nooffset" in STUB else wspan[g]
                            nc.gpsimd.dma_gather(
                                out_ap=vg[:, 3 * (bi_ - blo):
                                          3 * (bi_ - blo) + 3, :],
                                in_ap=vd[off:off + span, :],
                                idxs_ap=gxt[:, bi_:bi_ + 1, :],
                                num_idxs=CAP,
                                num_idxs_reg=nidx_regs[CAP],
                                elem_size=COUT,
                            )
                    else:
                        off = 0 if "nooffset" in STUB else wlo[g]
                        span = 32768 if "nooffset" in STUB else wspan[g]
                        nc.gpsimd.dma_gather(
                            out_ap=vg[:, :ntb * 3, :],
                            in_ap=vd[off:off + span, :],
                            idxs_ap=gxt[:, blo:bhi, :],
                            num_idxs=nid,
                            num_idxs_reg=nidx_regs[nid],
                            elem_size=COUT,
                        )
                    for bi in range(blo, bhi):
                        cb = 3 * (bi - blo)
                        hws = [None, None]
                        if not USE_CUSTOM_DVE:
                            # stock-op hat build, batched over the 3 c-tiles
                            tmp = hatpool.tile([128, 3, OWW], bf, tag="tmp")
                            nc.vector.tensor_tensor(
                                out=tmp[:],
                                in0=ioF1[:, None, :].to_broadcast(
                                    [128, 3, OWW]),
                                in1=xpat[:, bi].to_broadcast([128, 3, OWW]),
                                op=AL.subtract)
                            tmpi = tmp[:].bitcast(i16)
                            nc.vector.tensor_scalar(
                                out=tmpi, in0=tmpi, scalar1=0x7FFF,
                                scalar2=None, op0=AL.bitwise_and)
                            nc.vector.tensor_scalar(
                                out=tmp[:], in0=tmp[:], scalar1=1.0,
                                op0=AL.subtract, scalar2=0.0, op1=AL.min)
                            for corner in range(2):
                                hw = hatpool.tile([128, 3, OWW], bf,
                                                  tag="hw")
                                nc.vector.tensor_tensor(
                                    out=hw[:], in0=tmp[:],
                                    in1=wyts[corner][:, bi].to_broadcast(
                                        [128, 3, OWW]),
                                    op=AL.mult)
                                hws[corner] = hw

                        def lhs(corner, c):
                            if USE_CUSTOM_DVE:
                                hat = hatpool.tile([128, OWW], bf, tag="hat")
                                nc.vector._custom_dve(
                                    hatop, out=hat[:], in0=iotaB[:],
                                    s0=xpat[:, bi, c],
                                    s1=wyts[corner][:, bi, c])
                                return hat[:]
                            return hws[corner][:, c, :]

                        # corner dy=0 -> row bi-1 (closes it)
                        if bi >= 1:
                            ps = rowps[bi - 1]
                            for c in range(3):
                                nc.tensor.matmul(
                                    out=ps[:], lhsT=lhs(0, c),
                                    rhs=vg[:, cb + c, :],
                                    start=False, stop=False)
                            emit_row(bi - 1)
                        # corner dy=1 -> row bi (opens it)
                        if bi <= OHH - 1:
                            ps = open_row(bi)
                            for c in range(3):
                                nc.tensor.matmul(
                                    out=ps[:], lhsT=lhs(1, c),
                                    rhs=vg[:, cb + c, :],
                                    start=(c == 0), stop=False)

                # interleaved schedule: pass_a(pt) then ready groups
                gnext = 0
                for pt in range(NPT):
                    pass_a(pt)
                    while gnext < NGRP and pt_needed[gnext] <= pt:
                        pass_b(gnext)
                        gnext += 1
                for g in range(gnext, NGRP):
                    pass_b(g)
                assert not rowps, f"unclosed rows {list(rowps)}"

    lower_extended_insts(nc)
    _split_multi_waits(nc)
    return nc


# ---------------------------------------------------------------------------
# Runner (compile/load once; dispatch cheaply)
# ---------------------------------------------------------------------------
class Runner:
    def __init__(self, params, reps=1):
        import jax
        import jax.numpy as jnp
        from jax.sharding import Mesh, PartitionSpec
        from jax.experimental.shard_map import shard_map
        from concourse.bass2jax import (
            _bass_exec_p, install_neuronx_cc_hook, partition_id_tensor,
        )

        install_neuronx_cc_hook()
        nc = build_nc(params, reps)
        self.nc = nc
        in_names, out_names, out_avals = [], [], []
        pname = nc.partition_id_tensor.name if nc.partition_id_tensor else None
        for alloc in nc.m.functions[0].allocations:
            if not isinstance(alloc, mybir.MemoryLocationSet):
                continue
            name = alloc.memorylocations[0].name
            if alloc.kind == "ExternalInput":
                if name != pname:
                    in_names.append(name)
            elif alloc.kind == "ExternalOutput":
                shape = tuple(alloc.tensor_shape)
                dtype = mybir.dt.np(alloc.dtype)
                out_avals.append(jax.core.ShapedArray(shape, dtype))
                out_names.append(name)
        self.in_names, self.out_names = in_names, out_names
        self.out_avals = out_avals
        n_params = len(in_names)
        all_in = in_names + out_names + ([pname] if pname else [])

        def _body(*args):
            operands = list(args)
            if pname:
                operands.append(partition_id_tensor())
            return tuple(_bass_exec_p.bind(
                *operands, out_avals=tuple(out_avals), in_names=tuple(all_in),
                out_names=tuple(out_names), lowering_input_output_aliases=(),
                sim_require_finite=True, sim_require_nnan=True, nc=nc))

        devices = jax.devices()[:B]
        mesh = Mesh(np.asarray(devices), ("core",))
        in_specs = (PartitionSpec("core"),) * (n_params + len(out_avals))
        out_specs = (PartitionSpec("core"),) * len(out_names)
        self._mesh = mesh
        self._shard_body = shard_map(
            _body, mesh=mesh, in_specs=in_specs, out_specs=out_specs,
            check_rep=False,
        )
        donate = tuple(range(n_params, n_params + len(out_avals)))
        self._jit = jax.jit(self._shard_body, donate_argnums=donate,
                            keep_unused=True)
        self._jax = jax
        from jax.sharding import NamedSharding
        sh = NamedSharding(mesh, PartitionSpec("core"))
        zshapes = [((B * av.shape[0], *av.shape[1:]), av.dtype)
                   for av in out_avals]

        def _mk_zeros():
            return tuple(jnp.zeros(s, d) for s, d in zshapes)

        self._mk_zeros = jax.jit(_mk_zeros, out_shardings=(sh,) * len(zshapes))

    def concat_inputs(self, in_maps):
        return [np.concatenate([np.asarray(m[n]) for m in in_maps], axis=0)
                for n in self.in_names]

    def __call__(self, concat_in):
        outs = self._jit(*concat_in, *self._mk_zeros())
        self._jax.block_until_ready(outs)
        return [
            {name: np.asarray(outs[i]).reshape(B, *self.out_avals[i].shape)[c]
             for i, name in enumerate(self.out_names)}
            for c in range(B)
        ]

    def make_timing_fn(self, concat_in):
        """Device-resident operands: warm calls measure dispatch + exec."""
        import jax
        from jax.sharding import NamedSharding, PartitionSpec

        sh = NamedSharding(self._mesh, PartitionSpec("core"))
        dev_args = [jax.device_put(a, sh) for a in concat_in]
        jf = self._jit
        jax.block_until_ready(jf(*dev_args, *self._mk_zeros()))

        def call():
            outs = jf(*dev_args, *self._mk_zeros())
            jax.block_until_ready(outs)
            return outs
        return call


_RUNNERS = {}


def get_runner(reps=1, params=None):
    key = (reps, params)
    if key not in _RUNNERS:
        _RUNNERS[key] = Runner(params, reps)
    return _RUNNERS[key]


_LAST_PARAMS = None


def kernel(x, weight, offset, mask, bias):
    global _LAST_PARAMS
    x = np.asarray(x, dtype=np.float32)
    weight = np.asarray(weight, dtype=np.float32)
    offset = np.asarray(offset, dtype=np.float32)
    mask = np.asarray(mask, dtype=np.float32)
    bias = np.asarray(bias, dtype=np.float32)

    in_maps, params = _prep_all(x, weight, offset, mask, bias)
    _LAST_PARAMS = params
    r = get_runner(1, params)
    results = r(r.concat_inputs(in_maps))
    out = np.empty((B, COUT, OHH, OWW), dtype=np.float32)
    for b in range(B):
        od = results[b]["out"].astype(np.float32)  # [oy, ox, cout]
        out[b] = od.transpose(2, 0, 1)
    return out
